# revision 2
# baseline (speedup 1.0000x reference)
"""Trainium2 Bass kernel for nn_HadamardTransform: Y = X @ H4096_normalized.

Algorithm: H4096 (Sylvester, normalized) factors exactly as the Kronecker
product H32n (x) H128n.  Each row x of X, reshaped row-major to R[32, 128],
transforms as  Y_mat = G @ R @ H128u  with G = 2^-6 * H32u (all of the
2^-6 normalization folded into the 32-side so H128u stays exactly +-1).

On-chip scheme per 128x128 tile T (4 consecutive rows, SBUF partition
p = 32*rr + i, free = j, where column c = 128*i + j):
  MM-A: psumA = T.T @ W1      (W1 = I4 (x) G, block-diagonal 128x128)
        -> psumA[j, (rr,i')] : the i-transform, emerging j-on-partitions
  MM-B: psumB = psumA.T @ H128u
        -> psumB[(rr,i'), j'] : the j-transform, natural output layout
No transposes are needed anywhere; the fixed matrices W1/H128u are the
moving operands, the per-tile data is the (self-loading fp32) stationary
operand.

Sharding: X's 8192 rows split into 8 contiguous shards of 1024 rows, one
per NeuronCore (pure data parallelism, no collectives).
"""

import sys

import numpy as np

try:
    import concourse.bass as bass
except ImportError:  # repo not on sys.path in a fresh grading dir
    sys.path.insert(0, "/opt/trn_rl_repo")
    import concourse.bass as bass

import concourse.mybir as mybir
import concourse.tile as tile
from concourse import bacc
from concourse.bass_utils import run_bass_kernel_spmd

N_CORES = 8
ROWS = 8192
N = 4096
ROWS_PER_CORE = ROWS // N_CORES  # 1024
ROWS_PER_GROUP = 32              # rows moved per DMA (512 KiB)
GROUPS = ROWS_PER_CORE // ROWS_PER_GROUP  # 32
F32 = mybir.dt.float32


def _hadamard_u(n: int) -> np.ndarray:
    """Unnormalized Sylvester Hadamard matrix (+-1 entries)."""
    H = np.array([[1.0]], dtype=np.float64)
    while H.shape[0] < n:
        H = np.block([[H, H], [H, -H]])
    return H


def _constants() -> tuple[np.ndarray, np.ndarray]:
    G = (2.0 ** -6) * _hadamard_u(32)          # fold full 2^-6 norm here
    W1 = np.kron(np.eye(4), G).astype(np.float32)   # [128,128] block-diag
    HJ = _hadamard_u(128).astype(np.float32)        # [128,128] exact +-1
    return W1, HJ


def _build_bass(loop_reps: int | None = None):
    """loop_reps: if set, wrap the whole body in a HW For_i loop that
    repeats it loop_reps times (timing harness only — adds ~2us/rep
    back-edge barrier, result unchanged since the same X is re-read)."""
    nc = bacc.Bacc("TRN2", target_bir_lowering=False, debug=False)

    X = nc.dram_tensor("X", [ROWS_PER_CORE, N], F32, kind="ExternalInput")
    W1 = nc.dram_tensor("W1", [128, 128], F32, kind="ExternalInput")
    HJ = nc.dram_tensor("HJ", [128, 128], F32, kind="ExternalInput")
    Y = nc.dram_tensor("Y", [ROWS_PER_CORE, N], F32, kind="ExternalOutput")

    # row r = 32*g + 4*a + b ; column c = 128*i + j
    # SBUF group tile: partition p = 32*b + i, free f = 128*a + j
    X_re = X[:].rearrange(
        "(g a b) (i j) -> g b i a j", a=8, b=4, i=32, j=128
    )
    Y_re = Y[:].rearrange(
        "(g a b) (i j) -> g b i a j", a=8, b=4, i=32, j=128
    )

    with tile.TileContext(nc) as tc:
        with (
            tc.tile_pool(name="consts", bufs=1) as cpool,
            tc.tile_pool(name="xin", bufs=6) as xpool,
            tc.tile_pool(name="yout", bufs=4) as ypool,
            tc.tile_pool(name="mid", bufs=4) as spool,
            tc.tile_pool(name="psA", bufs=3, space="PSUM") as psA,
            tc.tile_pool(name="psB", bufs=3, space="PSUM") as psB,
        ):
            w1 = cpool.tile([128, 128], F32)
            nc.sync.dma_start(out=w1[:], in_=W1[:])
            hj = cpool.tile([128, 128], F32)
            nc.sync.dma_start(out=hj[:], in_=HJ[:])

            def flush_b(state):
                """Emit the B-stage (MM-B x4 + ACT copy + maybe store)
                for a previously A-staged half-group."""
                if state is None:
                    return
                sa, yw_3d_, yw_, h_, g_ = state
                pb = psB.tile([128, 512], F32)
                for q in range(4):
                    nc.tensor.matmul(
                        pb[:, q * 128:(q + 1) * 128],
                        lhsT=sa[:, q * 128:(q + 1) * 128],
                        rhs=hj[:],
                        start=True,
                        stop=True,
                    )
                nc.scalar.copy(
                    out=yw_[:, h_ * 512:(h_ + 1) * 512], in_=pb[:]
                )
                if h_ == 1:
                    # stores ride the ACT HWDGE ring; loads own the SP ring
                    # (a shared FIFO ring head-of-line-blocks loads behind
                    # stores that wait on compute).
                    nc.scalar.dma_start(out=Y_re[g_], in_=yw_3d_)

            def emit_body():
              # 1-stage software pipeline: each half-group's MM-B block is
              # emitted after the NEXT half-group's MM-A block, so the PE
              # FIFO never stalls on the DVE PSUM->SBUF copy in between.
              prev = None
              for g in range(GROUPS):
                xw = xpool.tile([128, 1024], F32)
                # SBUF partition dim must stay a single dim0; DRAM side
                # enumerates (b, i, a, j) which matches (p, a, j) order.
                xw_3d = xw[:].rearrange("p (a j) -> p a j", a=8, j=128)
                nc.sync.dma_start(out=xw_3d, in_=X_re[g])
                yw = ypool.tile([128, 1024], F32)
                yw_3d = yw[:].rearrange("p (a j) -> p a j", a=8, j=128)
                for h in range(2):
                    pa = psA.tile([128, 512], F32)
                    for q in range(4):
                        rg = 4 * h + q
                        nc.tensor.matmul(
                            pa[:, q * 128:(q + 1) * 128],
                            lhsT=xw[:, rg * 128:(rg + 1) * 128],
                            rhs=w1[:],
                            start=True,
                            stop=True,
                        )
                    flush_b(prev)
                    sa = spool.tile([128, 512], F32)
                    nc.vector.tensor_copy(out=sa[:], in_=pa[:])
                    prev = (sa, yw_3d, yw, h, g)
              flush_b(prev)

            if loop_reps is None:
                emit_body()
            else:
                with tc.For_i(0, loop_reps, 1):
                    emit_body()

    nc.compile()
    return nc


_NC = None


def _get_nc():
    global _NC
    if _NC is None:
        _NC = _build_bass()
    return _NC


def make_in_maps(X: np.ndarray) -> list[dict]:
    W1, HJ = _constants()
    return [
        {
            "X": X[c * ROWS_PER_CORE:(c + 1) * ROWS_PER_CORE],
            "W1": W1,
            "HJ": HJ,
        }
        for c in range(N_CORES)
    ]


def run(X: np.ndarray, trace: bool = False):
    """Run the SPMD kernel on 8 cores; returns (Y, BassKernelResults)."""
    X = np.ascontiguousarray(np.asarray(X, dtype=np.float32))
    assert X.shape == (ROWS, N), X.shape
    nc = _get_nc()
    in_maps = make_in_maps(X)
    res = run_bass_kernel_spmd(
        nc, in_maps, list(range(N_CORES)), trace=trace
    )
    Y = np.concatenate(
        [res.results[c]["Y"] for c in range(N_CORES)], axis=0
    )
    return Y, res


def kernel(X, H=None, **_unused) -> np.ndarray:
    """Full-input entry point: X (8192, 4096) f32, H ignored (H is the
    deterministic normalized Hadamard matrix, synthesized on device)."""
    Y, _ = run(X, trace=False)
    return Y

